# revision 15
# baseline (speedup 1.0000x reference)
"""Trainium2 Bass kernel for the BalancedSpikingNetwork problem.

Strategy: model-parallel over neurons across 8 NeuronCores, with the spike
exchange GROUPED over two timesteps (the spike->current data dependency has
exactly 2 steps of latency, so {z(2k+1), z(2k+2)} can travel in ONE AllToAll
consumed by bursts 2k+2 and 2k+3). One collective per two steps halves the
number of ~6-8us Mesh-collective floors paid, and leaves the CC cores idle
between calls (queuing a collective while the previous one is in flight was
measured to inflate durations 6.2us -> 8-10us).

  - Each core owns 256 E-neurons + 64 I-neurons (padded to 384 = 3x128 rows).
  - Per step: 24 gathered spike chunks + 1 input chunk accumulate into a
    PSUM tile [64, 320] (batch-major); weights are tau-pre-scaled on host.
  - Spikes cross cores BATCH-PAIR PACKED in fp8: byte = z[:, c] + 4*z[:, c+32]
    (values {0,1,4,5} exact in fp8e4). Per-rank slab [128, 192]: cols 0-95 =
    step A, 96-191 = step B.
  - Unpack: 2 DVE ops per (rank-group, step): is_gt 3.0 -> high bit,
    g - 4*hi -> low bit, writing CONTIGUOUS [128, 384] f32r ranges (the
    stationary AP reassembles batch order as (h c) which is identity).
  - Replication for the AllToAll-as-AllGather is split across the sync and
    scalar HWDGE rings, with step-A's half staged early (during burst 2k+1).
  - PE warmers: tiny fp8 matmuls gated on DMA round-trips chained after the
    replication transfer keep HAM at K=8/8 through the collective window.
  - Rates accumulate in [neuron, batch] layout straight from the spike tile
    (pad rows hold garbage but are dropped at the final transpose).
"""

import os
import sys

for _p in ("/opt/trn_rl_repo", "/root/.axon_site/_ro/trn_rl_repo"):
    if _p not in sys.path:
        sys.path.append(_p)

import numpy as np
import ml_dtypes

import concourse.bass as bass
import concourse.mybir as mybir
import concourse.tile as tile
from concourse import bacc
from concourse.bass_utils import run_bass_kernel_spmd
from concourse.masks import make_identity

F32 = mybir.dt.float32
F32R = mybir.dt.float32r
FP8 = mybir.dt.float8e4
OP = mybir.AluOpType

NWARM = int(os.environ.get("NWARM", "5"))  # warmer chain length

B, T_FULL, IN = 64, 512, 128
N_E, N_I = 2048, 512
NCORES = 8
E_LOC = N_E // NCORES          # 256
I_LOC = N_I // NCORES          # 64
NLOC = E_LOC + I_LOC           # 320 real outputs per core
PADLOC = 384                   # padded to 3 chunks of 128
NCHUNK = PADLOC // 128         # 3 chunks per source core
GCHUNK = NCORES * NCHUNK       # 24 gathered spike chunks
KSRC = GCHUNK * 128            # 3072 gathered contraction rows
SLOT = NCHUNK * B              # 192 spike columns per core
PAIR = B // 2                  # 32 packed columns per chunk-block
PSLOT = NCHUNK * PAIR          # 96 packed columns per core per step
GSLOT = 2 * PSLOT              # 192 packed columns per core (two steps)
HGRP = NCORES // 2             # 4 ranks per gather group
HB = HGRP * NCHUNK             # 12 chunks per gather group

TAU_E = 1.0 / 20.0
TAU_I = 1.0 / 10.0
SYN_DEC = 1.0 - 1.0 / 5.0      # 0.8


def build_kernel(T: int):
    assert T % 2 == 0
    K = T // 2
    nc = bacc.Bacc(
        "TRN2", target_bir_lowering=False, debug=False, num_devices=NCORES
    )

    W_in = nc.dram_tensor("W", [KSRC, NLOC], F32R, kind="ExternalInput")
    WIN_in = nc.dram_tensor("WIN", [IN, NLOC], F32, kind="ExternalInput")
    XT_in = nc.dram_tensor("XT", [T, IN, B], F32, kind="ExternalInput")
    RATES_out = nc.dram_tensor("RATES", [B, NLOC], F32, kind="ExternalOutput")

    rg = [list(range(NCORES))]

    with tile.TileContext(nc) as tc:
        with (
            tc.tile_pool(name="persist", bufs=1) as pp,
            tc.tile_pool(name="step", bufs=2) as sp,
            tc.tile_pool(name="psum", bufs=2, space="PSUM") as psp,
            tc.tile_pool(name="tpsum", bufs=1, space="PSUM") as tpp,
            tc.tile_pool(name="dram", bufs=2, space="DRAM") as dp,
            tc.tile_pool(name="wdram", bufs=1, space="DRAM") as wdp,
        ):
            # --- persistent tiles ---
            w_sb = pp.tile([128, GCHUNK * NLOC], F32R)            # recurrent wts
            win_sb = pp.tile([128, NLOC], F32)                    # input weights
            v_sb = pp.tile([B, NLOC], F32)                        # membrane
            u_sb = pp.tile([B, NLOC], F32)                        # tau*syn current
            zp_sb = pp.tile([128, GSLOT], FP8)                    # packed 2 steps
            rates_nb = pp.tile([128, SLOT], F32)                  # counts [n, b]
            ident = pp.tile([B, B], F32)
            ident128 = pp.tile([128, 128], F32)
            wsb = [pp.tile([1, B], FP8, name=f"wsb{i}")
                   for i in range(NWARM)]                         # warmer data

            for k in range(GCHUNK):
                nc.sync.dma_start(
                    out=w_sb[:, k * NLOC : (k + 1) * NLOC],
                    in_=W_in[k * 128 : (k + 1) * 128, :],
                )
            nc.sync.dma_start(out=win_sb, in_=WIN_in[:])
            make_identity(nc, ident)
            make_identity(nc, ident128)
            nc.vector.memset(v_sb, 0.0)
            nc.vector.memset(u_sb, 0.0)
            nc.vector.memset(rates_nb, 0.0)

            ag_prev = None   # gathered packed slabs {z(2k-1), z(2k)}

            def boundary(t, psum_cur, zp_half):
                """State update crossing into step t (reads psum(t-1)):
                v_dec(t), spikes z(t) -> zt/pack into zp_half, v/u/rates."""
                u08 = sp.tile([B, NLOC], F32, tag="U8")
                nc.scalar.mul(u08, u_sb, SYN_DEC)
                va = sp.tile([B, NLOC], F32, tag="VA")
                nc.scalar.mul(va[:, :E_LOC], v_sb[:, :E_LOC], 1.0 - TAU_E)
                nc.scalar.mul(va[:, E_LOC:], v_sb[:, E_LOC:], 1.0 - TAU_I)
                t1 = sp.tile([B, NLOC], F32, tag="T1")
                nc.gpsimd.tensor_tensor(out=t1, in0=va, in1=u08, op=OP.add)
                v_dec = sp.tile([B, NLOC], F32, tag="VD")
                nc.vector.tensor_tensor(
                    out=v_dec, in0=t1, in1=psum_cur[:], op=OP.add
                )
                # spikes in [n, b]: 3 transposes into ONE psum tile + is_gt
                tpq = tpp.tile([128, SLOT], F32, tag="TPQ")
                for j in range(NCHUNK):
                    w = 128 if j < 2 else I_LOC
                    nc.tensor.transpose(
                        tpq[:w, j * B : (j + 1) * B],
                        v_dec[:, j * 128 : j * 128 + w], ident,
                    )
                zt = sp.tile([128, SLOT], FP8, tag="ZT")
                nc.vector.tensor_scalar(
                    out=zt, in0=tpq, scalar1=1.0, scalar2=None, op0=OP.is_gt,
                )
                if zp_half is not None:
                    # pack batch pairs: byte = z[:, c] + 4*z[:, c+32]
                    zt_v = zt[:].rearrange("p (j h c) -> p j h c", h=2, c=PAIR)
                    nc.vector.scalar_tensor_tensor(
                        out=zp_half.rearrange("p (j c) -> p j c", c=PAIR),
                        in0=zt_v[:, :, 1, :], scalar=4.0,
                        in1=zt_v[:, :, 0, :], op0=OP.mult, op1=OP.add,
                    )
                # rates in [n, b] (pad rows accumulate garbage, dropped later)
                nc.gpsimd.tensor_tensor(
                    out=rates_nb, in0=rates_nb, in1=zt, op=OP.add
                )
                # v(t) = (v_dec <= 1) * v_dec
                nc.vector.scalar_tensor_tensor(
                    out=v_sb, in0=v_dec, scalar=1.0, in1=v_dec,
                    op0=OP.is_le, op1=OP.mult,
                )
                # u(t-1) = 0.8*u(t-2) + psum(t-1), via (v_dec - t1)
                u_tmp = sp.tile([B, NLOC], F32, tag="UT")
                nc.gpsimd.tensor_tensor(
                    out=u_tmp, in0=v_dec, in1=t1, op=OP.subtract
                )
                nc.gpsimd.tensor_tensor(
                    out=u_sb, in0=u08, in1=u_tmp, op=OP.add
                )

            def burst(t, s_tiles):
                """psum(t) = x_t @ Win + z(t-1) @ W, z(t-1) from s_tiles."""
                sx_t = sp.tile([128, B], F32, tag="SX")
                nc.sync.dma_start(out=sx_t, in_=XT_in[t])
                psum = psp.tile([B, NLOC], F32, tag="PS")
                nc.tensor.matmul(
                    psum, sx_t, win_sb, start=True, stop=(s_tiles is None)
                )
                if s_tiles is not None:
                    for k in range(GCHUNK):
                        s_t = s_tiles[k // HB]
                        kk = k % HB
                        nc.tensor.matmul(
                            psum,
                            s_t[:, kk * B : (kk + 1) * B],
                            w_sb[:, k * NLOC : (k + 1) * NLOC],
                            start=False, stop=(k == GCHUNK - 1),
                        )
                return psum

            for it in range(K):
                ta, tb = 2 * it, 2 * it + 1
                # ---- gather + unpack the previous exchange ----
                if ag_prev is not None:
                    g_sb = sp.tile([128, NCORES * GSLOT], FP8, tag="G")
                    for gi, eng in ((0, nc.sync), (1, nc.scalar)):
                        eng.dma_start(
                            out=g_sb[:, gi * HGRP * GSLOT :
                                     (gi + 1) * HGRP * GSLOT].rearrange(
                                "p (d c) -> p d c", d=HGRP),
                            in_=ag_prev[gi * HGRP * 128 :
                                        (gi + 1) * HGRP * 128].rearrange(
                                "(d p) c -> p d c", p=128),
                        )
                    # unpack to f32r stationaries: tiles indexed [step][grp]
                    s_ab = [[None, None], [None, None]]
                    for st in range(2):
                        for gi in range(2):
                            s_t = sp.tile(
                                [128, 2 * HB * PAIR], F32R, tag=f"S{st}{gi}")
                            sv = s_t[:].rearrange(
                                "p (d j h c) -> p d j h c",
                                d=HGRP, j=NCHUNK, c=PAIR)
                            gv = g_sb[:].rearrange(
                                "p (d s j c) -> p d s j c",
                                d=NCORES, s=2, j=NCHUNK, c=PAIR,
                            )[:, gi * HGRP : (gi + 1) * HGRP, st, :, :]
                            nc.vector.tensor_scalar(
                                out=sv[:, :, :, 1, :], in0=gv,
                                scalar1=3.0, scalar2=None, op0=OP.is_gt,
                            )
                            nc.vector.scalar_tensor_tensor(
                                out=sv[:, :, :, 0, :], in0=sv[:, :, :, 1, :],
                                scalar=-4.0, in1=gv, op0=OP.mult, op1=OP.add,
                            )
                            s_ab[st][gi] = s_t
                else:
                    s_ab = [None, None]

                # ---- burst(2k) consuming z(2k-1) = step-A half ----
                psum_a = burst(ta, s_ab[0])
                # ---- boundary 2k+1: z(2k+1) -> payload half A ----
                send = it < K - 1
                boundary(ta + 1, psum_a, zp_sb[:, :PSLOT] if send else None)
                if send:
                    a2a_in = dp.tile([NCORES * 128, GSLOT], FP8, tag="AGI")
                    # stage step-A half early (overlaps burst 2k+1):
                    # replicated 8x via stride-0 free dim, split over rings
                    for gi, eng in ((0, nc.sync), (1, nc.scalar)):
                        eng.dma_start(
                            out=a2a_in[gi * HGRP * 128 :
                                       (gi + 1) * HGRP * 128, :PSLOT
                                       ].rearrange("(d p) c -> p d c", p=128),
                            in_=zp_sb[:, :PSLOT].unsqueeze(1).broadcast_to(
                                [128, HGRP, PSLOT]),
                        )

                # ---- burst(2k+1) consuming z(2k) = step-B half ----
                if it < K - 1:
                    psum_b = burst(tb, s_ab[1])
                    # ---- boundary 2k+2: z(2k+2) -> payload half B ----
                    boundary(tb + 1, psum_b,
                             zp_sb[:, PSLOT:] if send else None)
                else:
                    psum_b = None

                if send:
                    for gi, eng in ((0, nc.sync), (1, nc.scalar)):
                        eng.dma_start(
                            out=a2a_in[gi * HGRP * 128 :
                                       (gi + 1) * HGRP * 128, PSLOT:
                                       ].rearrange("(d p) c -> p d c", p=128),
                            in_=zp_sb[:, PSLOT:].unsqueeze(1).broadcast_to(
                                [128, HGRP, PSLOT]),
                        )
                    ag_out = dp.tile([NCORES * 128, GSLOT], FP8, tag="AGO")
                    nc.gpsimd.collective_compute(
                        "AllToAll", OP.bypass, replica_groups=rg,
                        ins=[a2a_in[:]], outs=[ag_out[:]],
                    )
                    # warmer chain: DMA round-trips serialized after the
                    # repl transfer on the sync ring fire tiny PE matmuls
                    # spread through the collective window (keeps HAM warm)
                    for wi in range(NWARM):
                        if wi == 0:
                            nc.sync.dma_start(
                                out=wsb[0], in_=a2a_in[:1, :B])
                        else:
                            wd = wdp.tile([1, B], FP8, tag=f"WD{wi}")
                            nc.sync.dma_start(out=wd, in_=wsb[wi - 1])
                            nc.sync.dma_start(out=wsb[wi], in_=wd)
                        wps = tpp.tile([B, 8], F32, tag="WPS")
                        nc.tensor.matmul(
                            wps, wsb[wi][:, :B], wsb[wi][:, :8],
                            start=True, stop=True,
                        )
                    ag_prev = ag_out
                else:
                    ag_prev = None

            # ---- final: rates [n, b] -> [b, n] and store ----
            rates_sb = pp.tile([B, NLOC], F32)
            for j in range(NCHUNK):
                w = 128 if j < 2 else I_LOC
                rtp = tpp.tile([B, 128], F32, tag="RT")
                nc.tensor.transpose(
                    rtp[:, :w], rates_nb[:w, j * B : (j + 1) * B],
                    ident128[:w, :w],
                )
                nc.vector.tensor_copy(
                    rates_sb[:, j * 128 : j * 128 + w], rtp[:, :w]
                )
            nc.sync.dma_start(out=RATES_out[:], in_=rates_sb[:])

    nc.compile()
    return nc


def _prep_inputs(x, W_ee, W_ie, W_ei, W_ii, W_e_in, W_i_in):
    """Host-side: combined per-core weight matrices (tau-pre-scaled) +
    transposed input."""
    Wee = np.maximum(W_ee, 0).astype(np.float32)
    Wie = np.maximum(W_ie, 0).astype(np.float32)
    Wei = np.maximum(W_ei, 0).astype(np.float32)
    Wii = np.maximum(W_ii, 0).astype(np.float32)

    Ws, Wins = [], []
    for c in range(NCORES):
        Ec = slice(c * E_LOC, (c + 1) * E_LOC)
        Ic = slice(c * I_LOC, (c + 1) * I_LOC)
        Wc = np.zeros((KSRC, NLOC), np.float32)
        for d in range(NCORES):
            base = d * PADLOC
            Epre = slice(d * E_LOC, (d + 1) * E_LOC)
            Ipre = slice(d * I_LOC, (d + 1) * I_LOC)
            Wc[base : base + E_LOC, :E_LOC] = Wee[Ec, Epre].T
            Wc[base : base + E_LOC, E_LOC:] = Wie[Ic, Epre].T
            Wc[base + E_LOC : base + NLOC, :E_LOC] = -Wei[Ec, Ipre].T
            Wc[base + E_LOC : base + NLOC, E_LOC:] = -Wii[Ic, Ipre].T
        Wc[:, :E_LOC] *= TAU_E
        Wc[:, E_LOC:] *= TAU_I
        Ws.append(Wc)

        Wi = np.empty((IN, NLOC), np.float32)
        Wi[:, :E_LOC] = W_e_in[Ec].T * TAU_E
        Wi[:, E_LOC:] = W_i_in[Ic].T * TAU_I
        Wins.append(Wi)

    xT = np.ascontiguousarray(
        np.asarray(x, np.float32).transpose(1, 2, 0)
    )  # [T, IN, B]
    return Ws, Wins, xT


_CACHE = {}


def _get_kernel(T):
    if T not in _CACHE:
        _CACHE[T] = build_kernel(T)
    return _CACHE[T]


def run_spikes(x, W_ee, W_ie, W_ei, W_ii, W_e_in, W_i_in, T=None, trace=False):
    """Run the device portion; returns spike-count sums [B, N_E] and results."""
    T = x.shape[1] if T is None else T
    Ws, Wins, xT = _prep_inputs(x, W_ee, W_ie, W_ei, W_ii, W_e_in, W_i_in)
    xT = xT[:T]
    nc = _get_kernel(T)
    in_maps = [{"W": Ws[c], "WIN": Wins[c], "XT": xT} for c in range(NCORES)]
    res = run_bass_kernel_spmd(
        nc, in_maps, core_ids=list(range(NCORES)), trace=trace
    )
    R = np.stack([res.results[c]["RATES"] for c in range(NCORES)])  # [c, b, 320]
    counts = (
        R[:, :, :E_LOC].transpose(1, 0, 2).reshape(B, N_E)
    )  # [b, c*256 + n]
    return counts, res


def kernel(x, W_ee, W_ie, W_ei, W_ii, W_e_in, W_i_in, readout_w, readout_b):
    counts, _ = run_spikes(x, W_ee, W_ie, W_ei, W_ii, W_e_in, W_i_in)
    rates = counts / np.float32(x.shape[1])
    y = rates.astype(np.float32) @ np.asarray(readout_w, np.float32).T
    return (y + np.asarray(readout_b, np.float32)).astype(np.float32)


# revision 17
# speedup vs baseline: 1.6588x; 1.6588x over previous
"""Trainium2 Bass kernel for the BalancedSpikingNetwork problem.

Strategy: model-parallel over neurons across 8 NeuronCores.
  - Each core owns 256 E-neurons + 64 I-neurons (padded to 384 = 3x128 rows).
  - Per step: 24 gathered spike chunks + 1 local input chunk accumulate into a
    PSUM tile [64, 320] = tau-scaled input currents for this core's neurons
    (batch-major). Weights are pre-scaled by tau on the host.
  - Recurrent matmuls run in float32r (single-pass fp32, exact for 0/1
    spikes); spikes cross cores as fp8 (0/1 is exact), upconverted to f32r
    in 4 ACT-engine chunks that pipeline with the matmul burst.
  - Spike exchange: AllToAll with an 8x-replicated input slab (single-phase
    Mesh ~6.2us; an 8-rank AllGather lowers to 3-stage RDH ~12.7us on this
    runtime). The replication is split across the sync and scalar HWDGE
    rings (dest ranks 0-3 / 4-7).
  - ORDERING IS THE POINT: the gather + upconvert of the PREVIOUS exchange
    sit at the TOP of the loop body, so on the sync/scalar rings they are
    queued AHEAD of this step's replication (which waits for z(t), i.e. for
    the previous burst). With the v1 order the gather was FIFO-stalled
    behind the replication and the collective could never overlap the
    burst; with this order the even/odd-step chains interleave: A2A(t)
    carrying z(t) runs while the burst of step t computes, and is consumed
    by burst(t+1).
  - A tiny fp8 warmer matmul gated on a DMA read-back of the A2A input slab
    (FIFO-ordered after the replication transfer) splits the PE-idle window
    so HAM keeps the PE at 2.4 GHz.
  - LIF update with fused scalar_tensor_tensor ops on DVE; spike-rate sums
    accumulate in [batch, neuron] layout; final readout matmul on host.

The spike at step t depends only on state through t-1 (z(t) needs psum(t-1)
but not psum(t)), which is what makes the one-step overlap legal.
"""

import os
import sys

for _p in ("/opt/trn_rl_repo", "/root/.axon_site/_ro/trn_rl_repo"):
    if _p not in sys.path:
        sys.path.append(_p)

import numpy as np
import ml_dtypes

import concourse.bass as bass
import concourse.mybir as mybir
import concourse.tile as tile
from concourse import bacc
from concourse.bass_utils import run_bass_kernel_spmd
from concourse.masks import make_identity

F32 = mybir.dt.float32
F32R = mybir.dt.float32r
FP8 = mybir.dt.float8e4
OP = mybir.AluOpType
ACT_COPY = mybir.ActivationFunctionType.Copy

WARM = int(os.environ.get("WARM", "1"))

B, T_FULL, IN = 64, 512, 128
N_E, N_I = 2048, 512
NCORES = 8
E_LOC = N_E // NCORES          # 256
I_LOC = N_I // NCORES          # 64
NLOC = E_LOC + I_LOC           # 320 real outputs per core
PADLOC = 384                   # padded to 3 chunks of 128
NCHUNK = PADLOC // 128         # 3 chunks per source core
GCHUNK = NCORES * NCHUNK       # 24 gathered spike chunks
KSRC = GCHUNK * 128            # 3072 gathered contraction rows
SLOT = NCHUNK * B              # 192 staging columns per core
HGRP = NCORES // 2             # ranks per replication half

TAU_E = 1.0 / 20.0
TAU_I = 1.0 / 10.0
SYN_DEC = 1.0 - 1.0 / 5.0      # 0.8


def build_kernel(T: int):
    nc = bacc.Bacc(
        "TRN2", target_bir_lowering=False, debug=False, num_devices=NCORES
    )

    W_in = nc.dram_tensor("W", [KSRC, NLOC], F32R, kind="ExternalInput")
    WIN_in = nc.dram_tensor("WIN", [IN, NLOC], F32, kind="ExternalInput")
    XT_in = nc.dram_tensor("XT", [T, IN, B], F32, kind="ExternalInput")
    RATES_out = nc.dram_tensor("RATES", [B, NLOC], F32, kind="ExternalOutput")

    rg = [list(range(NCORES))]

    with tile.TileContext(nc) as tc:
        with (
            tc.tile_pool(name="persist", bufs=1) as pp,
            tc.tile_pool(name="step", bufs=2) as sp,
            tc.tile_pool(name="psum", bufs=2, space="PSUM") as psp,
            tc.tile_pool(name="tpsum", bufs=1, space="PSUM") as tpp,
            tc.tile_pool(name="dram", bufs=2, space="DRAM") as dp,
        ):
            # --- persistent tiles ---
            w_sb = pp.tile([128, GCHUNK * NLOC], F32R)            # recurrent wts
            win_sb = pp.tile([128, NLOC], F32)                    # input weights
            v_sb = pp.tile([B, NLOC], F32)                        # membrane
            u_sb = pp.tile([B, NLOC], F32)                        # tau*syn current
            zt_sb = pp.tile([128, SLOT], FP8)                     # spikes [n, b]
            rates_sb = pp.tile([B, NLOC], F32)                    # counts [b, n]
            ident = pp.tile([B, B], F32)
            wsb = pp.tile([1, B], FP8)                            # warmer data

            for k in range(GCHUNK):
                nc.sync.dma_start(
                    out=w_sb[:, k * NLOC : (k + 1) * NLOC],
                    in_=W_in[k * 128 : (k + 1) * 128, :],
                )
            nc.sync.dma_start(out=win_sb, in_=WIN_in[:])
            make_identity(nc, ident)
            nc.vector.memset(v_sb, 0.0)
            nc.vector.memset(u_sb, 0.0)
            nc.vector.memset(rates_sb, 0.0)

            ag_prev = None   # gathered spikes of step t-1
            psum_prev = None  # currents computed at step t-1

            for t in range(T):
                # ---- FIRST: gather + upconvert the previous exchange so
                # these transfers are queued on the sync/scalar rings AHEAD
                # of this step's replication (which waits for z(t)). ----
                s_t = None
                if t < T - 1 and ag_prev is not None:
                    s_raw = sp.tile([128, GCHUNK * B], FP8, tag="SR")
                    nc.sync.dma_start(
                        out=s_raw[:, : HGRP * SLOT].rearrange(
                            "p (d c) -> p d c", d=HGRP),
                        in_=ag_prev[: HGRP * 128].rearrange(
                            "(d p) c -> p d c", p=128),
                    )
                    nc.scalar.dma_start(
                        out=s_raw[:, HGRP * SLOT :].rearrange(
                            "p (d c) -> p d c", d=HGRP),
                        in_=ag_prev[HGRP * 128 :].rearrange(
                            "(d p) c -> p d c", p=128),
                    )
                    s_t = sp.tile([128, GCHUNK * B], F32R, tag="S")
                    q = GCHUNK * B // 4
                    for i in range(4):
                        nc.scalar.activation(
                            out=s_t[:, i * q : (i + 1) * q],
                            in_=s_raw[:, i * q : (i + 1) * q],
                            func=ACT_COPY,
                        )

                # ---- v_dec(t) = a*v(t-1) + 0.8*u(t-2) + psum(t-1).
                # t1 = a*v + 0.8*u uses only older state, so it overlaps the
                # previous burst; psum lands via ONE tensor_tensor add. ----
                u08 = sp.tile([B, NLOC], F32, tag="U8")
                nc.vector.tensor_scalar(
                    out=u08, in0=u_sb, scalar1=SYN_DEC, scalar2=None, op0=OP.mult
                )
                t1 = sp.tile([B, NLOC], F32, tag="T1")
                nc.vector.scalar_tensor_tensor(
                    out=t1[:, :E_LOC], in0=v_sb[:, :E_LOC], scalar=1.0 - TAU_E,
                    in1=u08[:, :E_LOC], op0=OP.mult, op1=OP.add,
                )
                nc.vector.scalar_tensor_tensor(
                    out=t1[:, E_LOC:], in0=v_sb[:, E_LOC:], scalar=1.0 - TAU_I,
                    in1=u08[:, E_LOC:], op0=OP.mult, op1=OP.add,
                )
                v_dec = sp.tile([B, NLOC], F32, tag="VD")
                if psum_prev is None:
                    nc.vector.tensor_copy(v_dec, t1)
                else:
                    nc.vector.tensor_tensor(
                        out=v_dec, in0=t1, in1=psum_prev, op=OP.add
                    )

                # ---- spikes in [n, b]: 3 transposes into ONE psum tile,
                # then a single is_gt (pad rows get 0/1 garbage that
                # multiplies zero weight columns; is_gt never yields NaN) ----
                tpq = tpp.tile([128, SLOT], F32, tag="TPQ")
                for j in range(NCHUNK):
                    w = 128 if j < 2 else I_LOC
                    nc.tensor.transpose(
                        tpq[:w, j * B : (j + 1) * B],
                        v_dec[:, j * 128 : j * 128 + w], ident,
                    )
                nc.vector.tensor_scalar(
                    out=zt_sb, in0=tpq, scalar1=1.0, scalar2=None, op0=OP.is_gt,
                )

                # ---- exchange spikes (overlaps the matmul burst below) ----
                if 1 <= t <= T - 3:
                    a2a_in = dp.tile([NCORES * 128, SLOT], FP8, tag="AGI")
                    # one-hop 8x replication: stride-0 FREE dim on the SBUF
                    # source, split across the two HWDGE rings
                    for gi, eng in ((0, nc.sync), (1, nc.scalar)):
                        eng.dma_start(
                            out=a2a_in[gi * HGRP * 128 :
                                       (gi + 1) * HGRP * 128].rearrange(
                                "(d p) c -> p d c", p=128),
                            in_=zt_sb[:].unsqueeze(1).broadcast_to(
                                [128, HGRP, SLOT]),
                        )
                    ag_out = dp.tile([NCORES * 128, SLOT], FP8, tag="AGO")
                    nc.gpsimd.collective_compute(
                        "AllToAll",
                        OP.bypass,
                        replica_groups=rg,
                        ins=[a2a_in[:]],
                        outs=[ag_out[:]],
                    )
                    if WARM:
                        # read-back FIFO-ordered after the sync-half of the
                        # replication: fires a tiny PE matmul mid-window
                        nc.sync.dma_start(out=wsb, in_=a2a_in[:1, :B])
                    new_ag = ag_out
                else:
                    new_ag = None if t == 0 else ag_prev

                # ---- u(t-1) = 0.8*u(t-2) + psum(t-1), off the chain ----
                if psum_prev is not None:
                    nc.vector.tensor_tensor(
                        out=u_sb, in0=u08, in1=psum_prev, op=OP.add
                    )

                # ---- input currents for step t (consumed at t+1) ----
                if t < T - 1:
                    sx_t = sp.tile([128, B], F32, tag="SX")
                    nc.sync.dma_start(out=sx_t, in_=XT_in[t])
                    if WARM and 1 <= t <= T - 3:
                        wps = tpp.tile([B, 8], F32, tag="WPS")
                        nc.tensor.matmul(
                            wps, wsb[:, :B], wsb[:, :8], start=True, stop=True
                        )
                    psum = psp.tile([B, NLOC], F32, tag="PS")
                    nc.tensor.matmul(
                        psum, sx_t, win_sb, start=True, stop=(s_t is None)
                    )
                    if s_t is not None:
                        for k in range(GCHUNK):
                            nc.tensor.matmul(
                                psum,
                                s_t[:, k * B : (k + 1) * B],
                                w_sb[:, k * NLOC : (k + 1) * NLOC],
                                start=False,
                                stop=(k == GCHUNK - 1),
                            )
                else:
                    psum = None
                ag_prev = new_ag

                # ---- rates accumulation in [b, n] layout ----
                zbn = sp.tile([B, NLOC], F32, tag="ZB")
                nc.vector.tensor_scalar(
                    out=zbn, in0=v_dec, scalar1=1.0, scalar2=None, op0=OP.is_gt
                )
                nc.gpsimd.tensor_tensor(
                    out=rates_sb, in0=rates_sb, in1=zbn, op=OP.add
                )

                # ---- v(t) = (v_dec <= 1) * v_dec ----
                nc.vector.scalar_tensor_tensor(
                    out=v_sb, in0=v_dec, scalar=1.0, in1=v_dec,
                    op0=OP.is_le, op1=OP.mult,
                )
                psum_prev = psum

            nc.sync.dma_start(out=RATES_out[:], in_=rates_sb[:])

    nc.compile()
    return nc


def _prep_inputs(x, W_ee, W_ie, W_ei, W_ii, W_e_in, W_i_in):
    """Host-side: combined per-core weight matrices (tau-pre-scaled) +
    transposed input."""
    Wee = np.maximum(W_ee, 0).astype(np.float32)
    Wie = np.maximum(W_ie, 0).astype(np.float32)
    Wei = np.maximum(W_ei, 0).astype(np.float32)
    Wii = np.maximum(W_ii, 0).astype(np.float32)

    Ws, Wins = [], []
    for c in range(NCORES):
        Ec = slice(c * E_LOC, (c + 1) * E_LOC)
        Ic = slice(c * I_LOC, (c + 1) * I_LOC)
        Wc = np.zeros((KSRC, NLOC), np.float32)
        for d in range(NCORES):
            base = d * PADLOC
            Epre = slice(d * E_LOC, (d + 1) * E_LOC)
            Ipre = slice(d * I_LOC, (d + 1) * I_LOC)
            Wc[base : base + E_LOC, :E_LOC] = Wee[Ec, Epre].T
            Wc[base : base + E_LOC, E_LOC:] = Wie[Ic, Epre].T
            Wc[base + E_LOC : base + NLOC, :E_LOC] = -Wei[Ec, Ipre].T
            Wc[base + E_LOC : base + NLOC, E_LOC:] = -Wii[Ic, Ipre].T
        Wc[:, :E_LOC] *= TAU_E
        Wc[:, E_LOC:] *= TAU_I
        Ws.append(Wc)

        Wi = np.empty((IN, NLOC), np.float32)
        Wi[:, :E_LOC] = W_e_in[Ec].T * TAU_E
        Wi[:, E_LOC:] = W_i_in[Ic].T * TAU_I
        Wins.append(Wi)

    xT = np.ascontiguousarray(
        np.asarray(x, np.float32).transpose(1, 2, 0)
    )  # [T, IN, B]
    return Ws, Wins, xT


_CACHE = {}


def _get_kernel(T):
    if T not in _CACHE:
        _CACHE[T] = build_kernel(T)
    return _CACHE[T]


def run_spikes(x, W_ee, W_ie, W_ei, W_ii, W_e_in, W_i_in, T=None, trace=False):
    """Run the device portion; returns spike-count sums [B, N_E] and results."""
    T = x.shape[1] if T is None else T
    Ws, Wins, xT = _prep_inputs(x, W_ee, W_ie, W_ei, W_ii, W_e_in, W_i_in)
    xT = xT[:T]
    nc = _get_kernel(T)
    in_maps = [{"W": Ws[c], "WIN": Wins[c], "XT": xT} for c in range(NCORES)]
    res = run_bass_kernel_spmd(
        nc, in_maps, core_ids=list(range(NCORES)), trace=trace
    )
    R = np.stack([res.results[c]["RATES"] for c in range(NCORES)])  # [c, b, 320]
    counts = (
        R[:, :, :E_LOC].transpose(1, 0, 2).reshape(B, N_E)
    )  # [b, c*256 + n]
    return counts, res


def kernel(x, W_ee, W_ie, W_ei, W_ii, W_e_in, W_i_in, readout_w, readout_b):
    counts, _ = run_spikes(x, W_ee, W_ie, W_ei, W_ii, W_e_in, W_i_in)
    rates = counts / np.float32(x.shape[1])
    y = rates.astype(np.float32) @ np.asarray(readout_w, np.float32).T
    return (y + np.asarray(readout_b, np.float32)).astype(np.float32)


# revision 20
# speedup vs baseline: 1.8464x; 1.1131x over previous
"""Trainium2 Bass kernel for the BalancedSpikingNetwork problem.

Strategy: model-parallel over neurons across 8 NeuronCores.
  - Each core owns 256 E-neurons + 64 I-neurons (padded to 384 = 3x128 rows).
  - Per step: 24 gathered spike chunks + 1 local input chunk accumulate into a
    PSUM tile [64, 320] = tau-scaled input currents for this core's neurons
    (batch-major). Weights are pre-scaled by tau on the host.
  - Recurrent matmuls run in float32r (single-pass fp32, exact for 0/1
    spikes); spikes cross cores as fp8 (0/1 is exact), upconverted to f32r
    on the VECTOR engine (keeping the scalar HWDGE ring free for DMA).
  - Spike exchange: AllToAll with an 8x-replicated input slab (single-phase
    Mesh; an 8-rank AllGather lowers to 3-stage RDH on this runtime).
  - SCHEDULING INVARIANT: the collective's Mesh transfer is extremely
    sensitive to concurrent HBM traffic (measured +3.5us when staging DMAs
    land mid-flight). All big DMAs (gather of the previous output, the
    split replication, the x_t load) are queued in ring-FIFO order so they
    execute in the serial window right after the previous collective
    completes, BEFORE the next doorbell rings; during the flight only
    SBUF-resident work runs (upconvert, matmul burst, LIF update, spike
    transpose/threshold).
  - Spike tiles are triple-buffered (own pool) so WAR dependencies never
    delay the gather into the next collective's flight window.

The spike at step t depends only on state through t-1 (z(t) needs psum(t-1)
but not psum(t)), so the exchange of z(t) overlaps the step-t matmul burst.
"""

import os
import sys

for _p in ("/opt/trn_rl_repo", "/root/.axon_site/_ro/trn_rl_repo"):
    if _p not in sys.path:
        sys.path.append(_p)

import numpy as np
import ml_dtypes

import concourse.bass as bass
import concourse.mybir as mybir
import concourse.tile as tile
from concourse import bacc
from concourse.bass_utils import run_bass_kernel_spmd
from concourse.masks import make_identity

F32 = mybir.dt.float32
F32R = mybir.dt.float32r
FP8 = mybir.dt.float8e4
OP = mybir.AluOpType
ACT_COPY = mybir.ActivationFunctionType.Copy

B, T_FULL, IN = 64, 512, 128
N_E, N_I = 2048, 512
NCORES = 8
E_LOC = N_E // NCORES          # 256
I_LOC = N_I // NCORES          # 64
NLOC = E_LOC + I_LOC           # 320 real outputs per core
PADLOC = 384                   # padded to 3 chunks of 128
NCHUNK = PADLOC // 128         # 3 chunks per source core
GCHUNK = NCORES * NCHUNK       # 24 gathered spike chunks
KSRC = GCHUNK * 128            # 3072 gathered contraction rows
SLOT = NCHUNK * B              # 192 staging columns per core
HGRP = NCORES // 2             # ranks per ring half

TAU_E = 1.0 / 20.0
TAU_I = 1.0 / 10.0
SYN_DEC = 1.0 - 1.0 / 5.0      # 0.8


def build_kernel(T: int):
    nc = bacc.Bacc(
        "TRN2", target_bir_lowering=False, debug=False, num_devices=NCORES
    )

    W_in = nc.dram_tensor("W", [KSRC, NLOC], F32R, kind="ExternalInput")
    WIN_in = nc.dram_tensor("WIN", [IN, NLOC], F32, kind="ExternalInput")
    XT_in = nc.dram_tensor("XT", [T, IN, B], F32, kind="ExternalInput")
    RATES_out = nc.dram_tensor("RATES", [B, NLOC], F32, kind="ExternalOutput")

    rg = [list(range(NCORES))]

    with tile.TileContext(nc) as tc:
        with (
            tc.tile_pool(name="persist", bufs=1) as pp,
            tc.tile_pool(name="step", bufs=2) as sp,
            tc.tile_pool(name="spk", bufs=3) as kp,
            tc.tile_pool(name="psum", bufs=2, space="PSUM") as psp,
            tc.tile_pool(name="tpsum", bufs=1, space="PSUM") as tpp,
            tc.tile_pool(name="dram", bufs=2, space="DRAM") as dp,
        ):
            # --- persistent tiles ---
            w_sb = pp.tile([128, GCHUNK * NLOC], F32R)            # recurrent wts
            win_sb = pp.tile([128, NLOC], F32)                    # input weights
            v_sb = pp.tile([B, NLOC], F32)                        # membrane
            u_sb = pp.tile([B, NLOC], F32)                        # tau*syn current
            rates_sb = pp.tile([B, NLOC], F32)                    # counts [b, n]
            ident = pp.tile([B, B], F32)

            for k in range(GCHUNK):
                nc.sync.dma_start(
                    out=w_sb[:, k * NLOC : (k + 1) * NLOC],
                    in_=W_in[k * 128 : (k + 1) * 128, :],
                )
            nc.sync.dma_start(out=win_sb, in_=WIN_in[:])
            make_identity(nc, ident)
            nc.vector.memset(v_sb, 0.0)
            nc.vector.memset(u_sb, 0.0)
            nc.vector.memset(rates_sb, 0.0)

            ag_prev = None   # gathered spikes of step t-1
            psum_prev = None  # currents computed at step t-1

            for t in range(T):
                # ---- FIRST on both HWDGE rings: gather the previous
                # exchange's output (the sem-gate on A2A(t-1) completion
                # pins these transfers into the post-flight serial window,
                # and everything queued behind them follows). ----
                s_t = None
                if t < T - 1 and ag_prev is not None:
                    s_raw = kp.tile([128, GCHUNK * B], FP8, tag="SR")
                    nc.sync.dma_start(
                        out=s_raw[:, : HGRP * SLOT].rearrange(
                            "p (d c) -> p d c", d=HGRP),
                        in_=ag_prev[: HGRP * 128].rearrange(
                            "(d p) c -> p d c", p=128),
                    )
                    nc.scalar.dma_start(
                        out=s_raw[:, HGRP * SLOT :].rearrange(
                            "p (d c) -> p d c", d=HGRP),
                        in_=ag_prev[HGRP * 128 :].rearrange(
                            "(d p) c -> p d c", p=128),
                    )

                # ---- v_dec(t) = a*v(t-1) + 0.8*u(t-2) + psum(t-1).
                # t1 = a*v + 0.8*u uses only older state, so it overlaps the
                # previous burst; psum lands via ONE tensor_tensor add. ----
                u08 = sp.tile([B, NLOC], F32, tag="U8")
                nc.vector.tensor_scalar(
                    out=u08, in0=u_sb, scalar1=SYN_DEC, scalar2=None, op0=OP.mult
                )
                t1 = sp.tile([B, NLOC], F32, tag="T1")
                nc.vector.scalar_tensor_tensor(
                    out=t1[:, :E_LOC], in0=v_sb[:, :E_LOC], scalar=1.0 - TAU_E,
                    in1=u08[:, :E_LOC], op0=OP.mult, op1=OP.add,
                )
                nc.vector.scalar_tensor_tensor(
                    out=t1[:, E_LOC:], in0=v_sb[:, E_LOC:], scalar=1.0 - TAU_I,
                    in1=u08[:, E_LOC:], op0=OP.mult, op1=OP.add,
                )
                v_dec = sp.tile([B, NLOC], F32, tag="VD")
                if psum_prev is None:
                    nc.vector.tensor_copy(v_dec, t1)
                else:
                    nc.vector.tensor_tensor(
                        out=v_dec, in0=t1, in1=psum_prev, op=OP.add
                    )

                # ---- spikes in [n, b] layout: transpose + threshold ----
                zt_sb = kp.tile([128, SLOT], FP8, tag="ZT")
                for j in range(NCHUNK):
                    w = 128 if j < 2 else I_LOC
                    tp = tpp.tile([128, B], F32, tag=f"TP{j}")
                    nc.tensor.transpose(
                        tp[:w, :], v_dec[:, j * 128 : j * 128 + w], ident
                    )
                    # full 128 rows: pad rows get 0/1 garbage that multiplies
                    # zero weight columns (is_gt never yields NaN)
                    nc.vector.tensor_scalar(
                        out=zt_sb[:, j * B : (j + 1) * B], in0=tp[:, :],
                        scalar1=1.0, scalar2=None, op0=OP.is_gt,
                    )

                # ---- exchange spikes: replication split across the two
                # rings, FIFO-behind the gathers -> doorbell rings only
                # after the serial DMA window drains ----
                if 1 <= t <= T - 3:
                    a2a_in = dp.tile([NCORES * 128, SLOT], FP8, tag="AGI")
                    for gi, eng in ((0, nc.sync), (1, nc.scalar)):
                        eng.dma_start(
                            out=a2a_in[gi * HGRP * 128 :
                                       (gi + 1) * HGRP * 128].rearrange(
                                "(d p) c -> p d c", p=128),
                            in_=zt_sb[:].unsqueeze(1).broadcast_to(
                                [128, HGRP, SLOT]),
                        )
                    ag_out = dp.tile([NCORES * 128, SLOT], FP8, tag="AGO")
                    nc.gpsimd.collective_compute(
                        "AllToAll",
                        OP.bypass,
                        replica_groups=rg,
                        ins=[a2a_in[:]],
                        outs=[ag_out[:]],
                    )
                    new_ag = ag_out
                else:
                    new_ag = None if t == 0 else ag_prev

                # ---- u(t-1) = 0.8*u(t-2) + psum(t-1), off the chain ----
                if psum_prev is not None:
                    nc.vector.tensor_tensor(
                        out=u_sb, in0=u08, in1=psum_prev, op=OP.add
                    )

                # ---- input currents for step t (consumed at t+1);
                # upconvert on DVE (SBUF-only, overlaps the flight) ----
                if t < T - 1:
                    sx_t = sp.tile([128, B], F32, tag="SX")
                    nc.sync.dma_start(out=sx_t, in_=XT_in[t])
                    psum = psp.tile([B, NLOC], F32, tag="PS")
                    nc.tensor.matmul(
                        psum, sx_t, win_sb, start=True, stop=(ag_prev is None)
                    )
                    if ag_prev is not None:
                        s_t = kp.tile([128, GCHUNK * B], F32R, tag="S")
                        q = GCHUNK * B // 4
                        for i in range(4):
                            nc.vector.tensor_scalar(
                                out=s_t[:, i * q : (i + 1) * q],
                                in0=s_raw[:, i * q : (i + 1) * q],
                                scalar1=1.0, scalar2=None, op0=OP.mult,
                            )
                        for k in range(GCHUNK):
                            nc.tensor.matmul(
                                psum,
                                s_t[:, k * B : (k + 1) * B],
                                w_sb[:, k * NLOC : (k + 1) * NLOC],
                                start=False,
                                stop=(k == GCHUNK - 1),
                            )
                else:
                    psum = None
                ag_prev = new_ag

                # ---- rates accumulation in [b, n] layout ----
                zbn = sp.tile([B, NLOC], F32, tag="ZB")
                nc.vector.tensor_scalar(
                    out=zbn, in0=v_dec, scalar1=1.0, scalar2=None, op0=OP.is_gt
                )
                nc.gpsimd.tensor_tensor(
                    out=rates_sb, in0=rates_sb, in1=zbn, op=OP.add
                )

                # ---- v(t) = (v_dec <= 1) * v_dec ----
                nc.vector.scalar_tensor_tensor(
                    out=v_sb, in0=v_dec, scalar=1.0, in1=v_dec,
                    op0=OP.is_le, op1=OP.mult,
                )
                psum_prev = psum

            nc.sync.dma_start(out=RATES_out[:], in_=rates_sb[:])

    nc.compile()
    return nc


def _prep_inputs(x, W_ee, W_ie, W_ei, W_ii, W_e_in, W_i_in):
    """Host-side: combined per-core weight matrices (tau-pre-scaled) +
    transposed input."""
    Wee = np.maximum(W_ee, 0).astype(np.float32)
    Wie = np.maximum(W_ie, 0).astype(np.float32)
    Wei = np.maximum(W_ei, 0).astype(np.float32)
    Wii = np.maximum(W_ii, 0).astype(np.float32)

    Ws, Wins = [], []
    for c in range(NCORES):
        Ec = slice(c * E_LOC, (c + 1) * E_LOC)
        Ic = slice(c * I_LOC, (c + 1) * I_LOC)
        Wc = np.zeros((KSRC, NLOC), np.float32)
        for d in range(NCORES):
            base = d * PADLOC
            Epre = slice(d * E_LOC, (d + 1) * E_LOC)
            Ipre = slice(d * I_LOC, (d + 1) * I_LOC)
            Wc[base : base + E_LOC, :E_LOC] = Wee[Ec, Epre].T
            Wc[base : base + E_LOC, E_LOC:] = Wie[Ic, Epre].T
            Wc[base + E_LOC : base + NLOC, :E_LOC] = -Wei[Ec, Ipre].T
            Wc[base + E_LOC : base + NLOC, E_LOC:] = -Wii[Ic, Ipre].T
        Wc[:, :E_LOC] *= TAU_E
        Wc[:, E_LOC:] *= TAU_I
        Ws.append(Wc)

        Wi = np.empty((IN, NLOC), np.float32)
        Wi[:, :E_LOC] = W_e_in[Ec].T * TAU_E
        Wi[:, E_LOC:] = W_i_in[Ic].T * TAU_I
        Wins.append(Wi)

    xT = np.ascontiguousarray(
        np.asarray(x, np.float32).transpose(1, 2, 0)
    )  # [T, IN, B]
    return Ws, Wins, xT


_CACHE = {}


def _get_kernel(T):
    if T not in _CACHE:
        _CACHE[T] = build_kernel(T)
    return _CACHE[T]


def run_spikes(x, W_ee, W_ie, W_ei, W_ii, W_e_in, W_i_in, T=None, trace=False):
    """Run the device portion; returns spike-count sums [B, N_E] and results."""
    T = x.shape[1] if T is None else T
    Ws, Wins, xT = _prep_inputs(x, W_ee, W_ie, W_ei, W_ii, W_e_in, W_i_in)
    xT = xT[:T]
    nc = _get_kernel(T)
    in_maps = [{"W": Ws[c], "WIN": Wins[c], "XT": xT} for c in range(NCORES)]
    res = run_bass_kernel_spmd(
        nc, in_maps, core_ids=list(range(NCORES)), trace=trace
    )
    R = np.stack([res.results[c]["RATES"] for c in range(NCORES)])  # [c, b, 320]
    counts = (
        R[:, :, :E_LOC].transpose(1, 0, 2).reshape(B, N_E)
    )  # [b, c*256 + n]
    return counts, res


def kernel(x, W_ee, W_ie, W_ei, W_ii, W_e_in, W_i_in, readout_w, readout_b):
    counts, _ = run_spikes(x, W_ee, W_ie, W_ei, W_ii, W_e_in, W_i_in)
    rates = counts / np.float32(x.shape[1])
    y = rates.astype(np.float32) @ np.asarray(readout_w, np.float32).T
    return (y + np.asarray(readout_b, np.float32)).astype(np.float32)


# revision 21
# speedup vs baseline: 3.3206x; 1.7984x over previous
"""Trainium2 Bass kernel for the BalancedSpikingNetwork problem.

Strategy: model-parallel over neurons across 8 NeuronCores.
  - Each core owns 256 E-neurons + 64 I-neurons (padded to 384 = 3x128 rows).
  - Per step: 24 gathered spike chunks + 1 local input chunk accumulate into a
    PSUM tile [64, 320] = tau-scaled input currents for this core's neurons
    (batch-major). Weights are pre-scaled by tau on the host.
  - Recurrent matmuls run in float32r (single-pass fp32, exact for 0/1
    spikes); spikes cross cores as fp8 (0/1 is exact), upconverted to f32r
    on the VECTOR engine (keeping the scalar HWDGE ring free for DMA).
  - Spike exchange: AllToAll with an 8x-replicated input slab (single-phase
    Mesh; an 8-rank AllGather lowers to 3-stage RDH on this runtime).
  - SCHEDULING INVARIANT: the collective's Mesh transfer is extremely
    sensitive to concurrent HBM traffic (measured +3.5us when staging DMAs
    land mid-flight). All big DMAs (gather of the previous output, the
    split replication, the x_t load) are queued in ring-FIFO order so they
    execute in the serial window right after the previous collective
    completes, BEFORE the next doorbell rings; during the flight only
    SBUF-resident work runs (upconvert, matmul burst, LIF update, spike
    transpose/threshold).
  - Spike tiles are triple-buffered (own pool) so WAR dependencies never
    delay the gather into the next collective's flight window.

The spike at step t depends only on state through t-1 (z(t) needs psum(t-1)
but not psum(t)), so the exchange of z(t) overlaps the step-t matmul burst.
"""

import os
import sys

for _p in ("/opt/trn_rl_repo", "/root/.axon_site/_ro/trn_rl_repo"):
    if _p not in sys.path:
        sys.path.append(_p)

import numpy as np
import ml_dtypes

import concourse.bass as bass
import concourse.mybir as mybir
import concourse.tile as tile
from concourse import bacc
from concourse.bass_utils import run_bass_kernel_spmd
from concourse.masks import make_identity

F32 = mybir.dt.float32
F32R = mybir.dt.float32r
FP8 = mybir.dt.float8e4
OP = mybir.AluOpType
ACT_COPY = mybir.ActivationFunctionType.Copy

B, T_FULL, IN = 64, 512, 128
N_E, N_I = 2048, 512
NCORES = 8
E_LOC = N_E // NCORES          # 256
I_LOC = N_I // NCORES          # 64
NLOC = E_LOC + I_LOC           # 320 real outputs per core
PADLOC = 384                   # padded to 3 chunks of 128
NCHUNK = PADLOC // 128         # 3 chunks per source core
GCHUNK = NCORES * NCHUNK       # 24 gathered spike chunks
KSRC = GCHUNK * 128            # 3072 gathered contraction rows
SLOT = NCHUNK * B              # 192 staging columns per core
PAIR = B // 2                  # 32 packed columns per chunk-block
PSLOT = NCHUNK * PAIR          # 96 packed columns per core
HGRP = NCORES // 2             # ranks per ring half
HB = GCHUNK // 2               # 12 chunks per unpack half

TAU_E = 1.0 / 20.0
TAU_I = 1.0 / 10.0
SYN_DEC = 1.0 - 1.0 / 5.0      # 0.8


def build_kernel(T: int):
    nc = bacc.Bacc(
        "TRN2", target_bir_lowering=False, debug=False, num_devices=NCORES
    )

    W_in = nc.dram_tensor("W", [KSRC, NLOC], F32R, kind="ExternalInput")
    WIN_in = nc.dram_tensor("WIN", [IN, NLOC], F32, kind="ExternalInput")
    XT_in = nc.dram_tensor("XT", [T, IN, B], F32, kind="ExternalInput")
    RATES_out = nc.dram_tensor("RATES", [B, NLOC], F32, kind="ExternalOutput")

    rg = [list(range(NCORES))]

    with tile.TileContext(nc) as tc:
        with (
            tc.tile_pool(name="persist", bufs=1) as pp,
            tc.tile_pool(name="step", bufs=2) as sp,
            tc.tile_pool(name="spk", bufs=3) as kp,
            tc.tile_pool(name="psum", bufs=2, space="PSUM") as psp,
            tc.tile_pool(name="tpsum", bufs=1, space="PSUM") as tpp,
            tc.tile_pool(name="dram", bufs=2, space="DRAM") as dp,
        ):
            # --- persistent tiles ---
            w_sb = pp.tile([128, GCHUNK * NLOC], F32R)            # recurrent wts
            win_sb = pp.tile([128, NLOC], F32)                    # input weights
            v_sb = pp.tile([B, NLOC], F32)                        # membrane
            u_sb = pp.tile([B, NLOC], F32)                        # tau*syn current
            rates_sb = pp.tile([B, NLOC], F32)                    # counts [b, n]
            ident = pp.tile([B, B], F32)

            for k in range(GCHUNK):
                nc.sync.dma_start(
                    out=w_sb[:, k * NLOC : (k + 1) * NLOC],
                    in_=W_in[k * 128 : (k + 1) * 128, :],
                )
            nc.sync.dma_start(out=win_sb, in_=WIN_in[:])
            make_identity(nc, ident)
            nc.vector.memset(v_sb, 0.0)
            nc.vector.memset(u_sb, 0.0)
            nc.vector.memset(rates_sb, 0.0)

            ag_prev = None   # gathered spikes of step t-1
            psum_prev = None  # currents computed at step t-1

            for t in range(T):
                # ---- FIRST on both HWDGE rings: gather the previous
                # exchange's output (the sem-gate on A2A(t-1) completion
                # pins these transfers into the post-flight serial window,
                # and everything queued behind them follows). ----
                s_t = None
                if t < T - 1 and ag_prev is not None:
                    s_raw = kp.tile([128, GCHUNK * PAIR], FP8, tag="SR")
                    nc.sync.dma_start(
                        out=s_raw[:, : HGRP * PSLOT].rearrange(
                            "p (d c) -> p d c", d=HGRP),
                        in_=ag_prev[: HGRP * 128].rearrange(
                            "(d p) c -> p d c", p=128),
                    )
                    nc.scalar.dma_start(
                        out=s_raw[:, HGRP * PSLOT :].rearrange(
                            "p (d c) -> p d c", d=HGRP),
                        in_=ag_prev[HGRP * 128 :].rearrange(
                            "(d p) c -> p d c", p=128),
                    )

                # ---- v_dec(t) = a*v(t-1) + 0.8*u(t-2) + psum(t-1).
                # t1 = a*v + 0.8*u uses only older state, so it overlaps the
                # previous burst; psum lands via ONE tensor_tensor add. ----
                u08 = sp.tile([B, NLOC], F32, tag="U8")
                nc.vector.tensor_scalar(
                    out=u08, in0=u_sb, scalar1=SYN_DEC, scalar2=None, op0=OP.mult
                )
                t1 = sp.tile([B, NLOC], F32, tag="T1")
                nc.vector.scalar_tensor_tensor(
                    out=t1[:, :E_LOC], in0=v_sb[:, :E_LOC], scalar=1.0 - TAU_E,
                    in1=u08[:, :E_LOC], op0=OP.mult, op1=OP.add,
                )
                nc.vector.scalar_tensor_tensor(
                    out=t1[:, E_LOC:], in0=v_sb[:, E_LOC:], scalar=1.0 - TAU_I,
                    in1=u08[:, E_LOC:], op0=OP.mult, op1=OP.add,
                )
                v_dec = sp.tile([B, NLOC], F32, tag="VD")
                if psum_prev is None:
                    nc.vector.tensor_copy(v_dec, t1)
                else:
                    nc.vector.tensor_tensor(
                        out=v_dec, in0=t1, in1=psum_prev, op=OP.add
                    )

                # ---- spikes in [n, b] layout: transpose + threshold ----
                zt_sb = kp.tile([128, SLOT], FP8, tag="ZT")
                for j in range(NCHUNK):
                    w = 128 if j < 2 else I_LOC
                    tp = tpp.tile([128, B], F32, tag=f"TP{j}")
                    nc.tensor.transpose(
                        tp[:w, :], v_dec[:, j * 128 : j * 128 + w], ident
                    )
                    # full 128 rows: pad rows get 0/1 garbage that multiplies
                    # zero weight columns (is_gt never yields NaN)
                    nc.vector.tensor_scalar(
                        out=zt_sb[:, j * B : (j + 1) * B], in0=tp[:, :],
                        scalar1=1.0, scalar2=None, op0=OP.is_gt,
                    )
                # pack batch pairs: byte = z[:, c] + 4*z[:, c+32]
                # (values {0,1,4,5} exact in fp8e4) -> halves both
                # serial-window DMAs and the A2A payload
                zp = kp.tile([128, PSLOT], FP8, tag="ZP")
                zt_v = zt_sb[:].rearrange("p (j h c) -> p j h c", h=2, c=PAIR)
                nc.vector.scalar_tensor_tensor(
                    out=zp[:].rearrange("p (j c) -> p j c", c=PAIR),
                    in0=zt_v[:, :, 1, :], scalar=4.0,
                    in1=zt_v[:, :, 0, :], op0=OP.mult, op1=OP.add,
                )

                # ---- exchange spikes: replication split across the two
                # rings, FIFO-behind the gathers -> doorbell rings only
                # after the serial DMA window drains ----
                if 1 <= t <= T - 3:
                    a2a_in = dp.tile([NCORES * 128, PSLOT], FP8, tag="AGI")
                    for gi, eng in ((0, nc.sync), (1, nc.scalar)):
                        eng.dma_start(
                            out=a2a_in[gi * HGRP * 128 :
                                       (gi + 1) * HGRP * 128].rearrange(
                                "(d p) c -> p d c", p=128),
                            in_=zp[:].unsqueeze(1).broadcast_to(
                                [128, HGRP, PSLOT]),
                        )
                    ag_out = dp.tile([NCORES * 128, PSLOT], FP8, tag="AGO")
                    nc.gpsimd.collective_compute(
                        "AllToAll",
                        OP.bypass,
                        replica_groups=rg,
                        ins=[a2a_in[:]],
                        outs=[ag_out[:]],
                    )
                    new_ag = ag_out
                else:
                    new_ag = None if t == 0 else ag_prev

                # ---- u(t-1) = 0.8*u(t-2) + psum(t-1), off the chain ----
                if psum_prev is not None:
                    nc.vector.tensor_tensor(
                        out=u_sb, in0=u08, in1=psum_prev, op=OP.add
                    )

                # ---- input currents for step t (consumed at t+1);
                # upconvert on DVE (SBUF-only, overlaps the flight) ----
                if t < T - 1:
                    sx_t = sp.tile([128, B], F32, tag="SX")
                    nc.sync.dma_start(out=sx_t, in_=XT_in[t])
                    psum = psp.tile([B, NLOC], F32, tag="PS")
                    nc.tensor.matmul(
                        psum, sx_t, win_sb, start=True, stop=(ag_prev is None)
                    )
                    if ag_prev is not None:
                        # unpack halves: is_gt 3.0 -> high bit, g - 4*hi ->
                        # low bit, f32r stationaries (batch order (h c) is
                        # identity)
                        s_half = []
                        for hh in range(2):
                            s_t = kp.tile([128, HB * B], F32R, tag=f"S{hh}")
                            sv = s_t[:].rearrange(
                                "p (m h c) -> p m h c", h=2, c=PAIR)
                            gv = s_raw[:, hh * HB * PAIR :
                                       (hh + 1) * HB * PAIR].rearrange(
                                "p (m c) -> p m c", c=PAIR)
                            nc.vector.tensor_scalar(
                                out=sv[:, :, 1, :], in0=gv,
                                scalar1=3.0, scalar2=None, op0=OP.is_gt,
                            )
                            nc.vector.scalar_tensor_tensor(
                                out=sv[:, :, 0, :], in0=sv[:, :, 1, :],
                                scalar=-4.0, in1=gv, op0=OP.mult, op1=OP.add,
                            )
                            s_half.append(s_t)
                        for k in range(GCHUNK):
                            s_t = s_half[k // HB]
                            kk = k % HB
                            nc.tensor.matmul(
                                psum,
                                s_t[:, kk * B : (kk + 1) * B],
                                w_sb[:, k * NLOC : (k + 1) * NLOC],
                                start=False,
                                stop=(k == GCHUNK - 1),
                            )
                else:
                    psum = None
                ag_prev = new_ag

                # ---- rates accumulation in [b, n] layout ----
                zbn = sp.tile([B, NLOC], F32, tag="ZB")
                nc.vector.tensor_scalar(
                    out=zbn, in0=v_dec, scalar1=1.0, scalar2=None, op0=OP.is_gt
                )
                nc.gpsimd.tensor_tensor(
                    out=rates_sb, in0=rates_sb, in1=zbn, op=OP.add
                )

                # ---- v(t) = (v_dec <= 1) * v_dec ----
                nc.vector.scalar_tensor_tensor(
                    out=v_sb, in0=v_dec, scalar=1.0, in1=v_dec,
                    op0=OP.is_le, op1=OP.mult,
                )
                psum_prev = psum

            nc.sync.dma_start(out=RATES_out[:], in_=rates_sb[:])

    nc.compile()
    return nc


def _prep_inputs(x, W_ee, W_ie, W_ei, W_ii, W_e_in, W_i_in):
    """Host-side: combined per-core weight matrices (tau-pre-scaled) +
    transposed input."""
    Wee = np.maximum(W_ee, 0).astype(np.float32)
    Wie = np.maximum(W_ie, 0).astype(np.float32)
    Wei = np.maximum(W_ei, 0).astype(np.float32)
    Wii = np.maximum(W_ii, 0).astype(np.float32)

    Ws, Wins = [], []
    for c in range(NCORES):
        Ec = slice(c * E_LOC, (c + 1) * E_LOC)
        Ic = slice(c * I_LOC, (c + 1) * I_LOC)
        Wc = np.zeros((KSRC, NLOC), np.float32)
        for d in range(NCORES):
            base = d * PADLOC
            Epre = slice(d * E_LOC, (d + 1) * E_LOC)
            Ipre = slice(d * I_LOC, (d + 1) * I_LOC)
            Wc[base : base + E_LOC, :E_LOC] = Wee[Ec, Epre].T
            Wc[base : base + E_LOC, E_LOC:] = Wie[Ic, Epre].T
            Wc[base + E_LOC : base + NLOC, :E_LOC] = -Wei[Ec, Ipre].T
            Wc[base + E_LOC : base + NLOC, E_LOC:] = -Wii[Ic, Ipre].T
        Wc[:, :E_LOC] *= TAU_E
        Wc[:, E_LOC:] *= TAU_I
        Ws.append(Wc)

        Wi = np.empty((IN, NLOC), np.float32)
        Wi[:, :E_LOC] = W_e_in[Ec].T * TAU_E
        Wi[:, E_LOC:] = W_i_in[Ic].T * TAU_I
        Wins.append(Wi)

    xT = np.ascontiguousarray(
        np.asarray(x, np.float32).transpose(1, 2, 0)
    )  # [T, IN, B]
    return Ws, Wins, xT


_CACHE = {}


def _get_kernel(T):
    if T not in _CACHE:
        _CACHE[T] = build_kernel(T)
    return _CACHE[T]


def run_spikes(x, W_ee, W_ie, W_ei, W_ii, W_e_in, W_i_in, T=None, trace=False):
    """Run the device portion; returns spike-count sums [B, N_E] and results."""
    T = x.shape[1] if T is None else T
    Ws, Wins, xT = _prep_inputs(x, W_ee, W_ie, W_ei, W_ii, W_e_in, W_i_in)
    xT = xT[:T]
    nc = _get_kernel(T)
    in_maps = [{"W": Ws[c], "WIN": Wins[c], "XT": xT} for c in range(NCORES)]
    res = run_bass_kernel_spmd(
        nc, in_maps, core_ids=list(range(NCORES)), trace=trace
    )
    R = np.stack([res.results[c]["RATES"] for c in range(NCORES)])  # [c, b, 320]
    counts = (
        R[:, :, :E_LOC].transpose(1, 0, 2).reshape(B, N_E)
    )  # [b, c*256 + n]
    return counts, res


def kernel(x, W_ee, W_ie, W_ei, W_ii, W_e_in, W_i_in, readout_w, readout_b):
    counts, _ = run_spikes(x, W_ee, W_ie, W_ei, W_ii, W_e_in, W_i_in)
    rates = counts / np.float32(x.shape[1])
    y = rates.astype(np.float32) @ np.asarray(readout_w, np.float32).T
    return (y + np.asarray(readout_b, np.float32)).astype(np.float32)
